# revision 1
# baseline (speedup 1.0000x reference)
"""Trainium2 Bass kernel v2 for nn_Net_3152505995417 (gnn_message_passing).

Same closed-form pair-dense reformulation as the baseline (see kernel.py),
rebuilt around measured HW costs:
  - bf16 tiles everywhere -> single-pass PE matmuls (4x fewer PE cycles)
    and 2x DVE mode on the fat [116,580] tensor_tensor passes.
  - We / p2 shipped from host as pre-broadcast [116,580] tiles (WeX, P2X)
    so the k->m MAC products are stride-1 TTs (2x) instead of 1x
    broadcast-AP passes; d2 collapses to one TT + one strided reduce.
  - colmax reciprocal via per-node 1/(max(dv,0)+eps) ([116,1] recip) and
    r[i,j] = min(ric_i, ric_j) -- replaces the 871ns full-tile reciprocal.
  - S = rowsum(Gn) via one mid-axis tensor_reduce; u-row via PE colsum of
    the symmetric Gn + gpsimd multiply; z = q + U with U a rank-1 PE
    broadcast; e2 bias (t1b) rides the per-plane scalar-engine ACT.
  - pool+head folded: out = (1'A2 (x1 W2 Wl))/N + b2'Wl + bl, so the tail
    is one rowsum matmul + one [116,4]x[116,1] matmul.
Replicated on all 8 cores; core 0's output is returned.
"""

import numpy as np

N = 116
E = N * (N - 1) // 2
HID = 64
EDIM = 5
OUT = 4
ENC = HID + N // 2
EPS = 1e-10
SLAB = EDIM * N  # 580

# ---- s16 (bf16) column map ----
C_EA = 0                      # [116, 0:580] pair-dense edge attrs, k-major
C_ENCT = 580                  # [122, 580:696]
C_WENC = 696                  # [122, 696:760]
C_W1 = 760                    # [64, 760:824]
C_W2T = 824                   # [64, 824:888]  W2 transposed
C_WL = 888                    # [64, 888:892]
C_SV = 892                    # [1, 892:932]  p1|p2|be|We flat
C_ONESR = 932                 # [1, 932:1048] ones row
C_ONESC = 1048                # [116, 1048:1049] ones col
C_PET = 1049                  # [64, 1049:1050] pe as column
C_B2 = 1050                   # [64, 1050:1051] b2 as column
B1_LO, B1_HI = 580, 1051
C_I = 1051                    # [116, 1051:1167] identity
C_MASK = 1167                 # [116, 1167:1283] 1-eye
B2_LO, B2_HI = 1051, 1283
C_WEX = 1283                  # [116, 1283+580k : ...] We[k,m] bcast, k=0..3
S16_W = C_WEX + 4 * SLAB      # 3603

# s32 (fp32) columns
C32_BENC = 0                  # [64,1]
C32_B1 = 1
C32_BL = 2                    # [4,1]
C32_SV = 4                    # [116, 4:44] p1|p2|be|We broadcast to all rows
S32_W = 44

CH1 = (0, 2 * N)              # PSUM chunking of the 580 slab (planes 0-1)
CH2 = (2 * N, SLAB)           # planes 2-4

_CACHE = {}


def _split_excess_waits(nc, mybir, max_waits=1):
    """Walrus on this build accepts only one sync-wait per instruction;
    move excess waits onto chained NoOps on the same engine."""
    for fn in nc.m.functions:
        for blk in fn.blocks:
            insts = blk.instructions
            new, changed = [], False
            for ins in insts:
                si = ins.sync_info
                waits = list(si.on_wait) if si is not None else []
                if len(waits) > max_waits:
                    while len(waits) > max_waits:
                        chunk, waits = waits[:1], waits[1:]
                        nop = mybir.InstNoOp(
                            name=nc.get_next_instruction_name(),
                            engine=ins.engine,
                            sync_info=mybir.SyncInfo(on_wait=chunk, on_update=[]),
                            bass_nofuse=True,
                        )
                        new.append(nop)
                    si.on_wait = waits
                    changed = True
                new.append(ins)
            if changed:
                blk.instructions = new


def _build():
    import concourse.bass as bass
    import concourse.tile as tile
    from concourse import mybir

    f32 = mybir.dt.float32
    bf16 = mybir.dt.bfloat16
    A = mybir.AluOpType
    Relu = mybir.ActivationFunctionType.Relu
    Ident = mybir.ActivationFunctionType.Identity

    nc = bass.Bass("TRN2", target_bir_lowering=False, num_devices=8)

    s16_d = nc.declare_dram_parameter("s16", [128, S16_W], bf16, isOutput=False)
    s32_d = nc.declare_dram_parameter("s32", [128, S32_W], f32, isOutput=False)
    out_d = nc.declare_dram_parameter("out", [OUT, 1], f32, isOutput=True)

    with tile.TileContext(nc) as tc:
        with (
            tc.tile_pool(name="sb", bufs=1) as sb,
            tc.tile_pool(name="pm", bufs=4, space="PSUM") as pm,
            tc.tile_pool(name="pw", bufs=1, space="PSUM") as pw,
            tc.tile_pool(name="pu", bufs=1, space="PSUM") as pu,
        ):
            t16 = sb.tile([128, S16_W], bf16, tag="t16")
            t32 = sb.tile([128, S32_W], f32, tag="t32")

            # ---- input DMAs (issue order per engine = queue order) ----
            nc.sync.dma_start(out=t32[:], in_=s32_d[:])
            nc.sync.dma_start(out=t16[:, B1_LO:B1_HI], in_=s16_d[:, B1_LO:B1_HI])
            nc.sync.dma_start(
                out=t16[:, C_WEX:C_WEX + SLAB], in_=s16_d[:, C_WEX:C_WEX + SLAB]
            )
            nc.sync.dma_start(
                out=t16[:, C_WEX + 3 * SLAB:C_WEX + 4 * SLAB],
                in_=s16_d[:, C_WEX + 3 * SLAB:C_WEX + 4 * SLAB],
            )
            nc.scalar.dma_start(out=t16[:, 290:580], in_=s16_d[:, 290:580])
            nc.scalar.dma_start(
                out=t16[:, C_WEX + SLAB:C_WEX + 2 * SLAB],
                in_=s16_d[:, C_WEX + SLAB:C_WEX + 2 * SLAB],
            )
            nc.gpsimd.dma_start(out=t16[:, 0:290], in_=s16_d[:, 0:290])
            nc.gpsimd.dma_start(out=t16[:, B2_LO:B2_HI], in_=s16_d[:, B2_LO:B2_HI])
            nc.gpsimd.dma_start(
                out=t16[:, C_WEX + 2 * SLAB:C_WEX + 3 * SLAB],
                in_=s16_d[:, C_WEX + 2 * SLAB:C_WEX + 3 * SLAB],
            )

            # ---- named views ----
            ea = t16[0:N, C_EA:C_EA + SLAB]
            encT = t16[0:ENC, C_ENCT:C_ENCT + N]
            Wenc = t16[0:ENC, C_WENC:C_WENC + HID]
            W1 = t16[0:HID, C_W1:C_W1 + HID]
            W2T = t16[0:HID, C_W2T:C_W2T + HID]
            Wl = t16[0:HID, C_WL:C_WL + OUT]
            sv = t16[0:1, C_SV:C_SV + 40]
            ones_row = t16[0:1, C_ONESR:C_ONESR + N]
            ones_col = t16[0:N, C_ONESC:C_ONESC + 1]
            peT = t16[0:HID, C_PET:C_PET + 1]
            b216 = t16[0:HID, C_B2:C_B2 + 1]
            I116 = t16[0:N, C_I:C_I + N]
            p2c16 = t16[0:N, C_MASK:C_MASK + EDIM]

            def WeX(k):
                return t16[0:N, C_WEX + k * SLAB:C_WEX + (k + 1) * SLAB]

            benc = t32[0:HID, C32_BENC:C32_BENC + 1]
            b1 = t32[0:HID, C32_B1:C32_B1 + 1]
            bl = t32[0:OUT, C32_BL:C32_BL + 1]

            def m3(ap):
                return ap.rearrange("p (m j) -> p m j", m=EDIM)

            # ---- scalar engine: warm the ACT table early ----
            warm_src = sb.tile([1, 1], f32, tag="warm_src")
            nc.vector.memset(warm_src[:], 1.0)
            warm = sb.tile([1, 1], f32, tag="warm")
            nc.scalar.activation(warm[:], warm_src[:], Relu)

            svecB = t32[0:N, C32_SV:C32_SV + 40]
            p1B = svecB[:, 0:5]
            beB = svecB[:, 10:15]

            # ---- x = enc @ W_enc + b_enc (kept transposed [HID, N]) ----
            xT_ps = pm.tile([HID, N], f32, tag="ps")
            nc.tensor.matmul(xT_ps[:], Wenc, encT, start=True, stop=True)
            xT = sb.tile([HID, N], bf16, tag="xT")
            nc.scalar.activation(xT[:], xT_ps[:], Ident, bias=benc)

            # ---- d1 (5-term MAC, in-place accumulation) ----
            d1 = sb.tile([N, N], bf16, tag="d1")
            nc.vector.tensor_scalar(d1[:], ea[:, 0:N], p1B[:, 0:1], None, A.mult)
            for k in range(1, EDIM):
                nc.vector.scalar_tensor_tensor(
                    d1[:], ea[:, k * N:(k + 1) * N], p1B[:, k:k + 1], d1[:],
                    A.mult, A.add,
                )

            # ---- W2l = W2 @ Wl; bbl = Wl^T b2 + bl ----
            W2l_ps = pm.tile([HID, OUT], f32, tag="ps")
            nc.tensor.matmul(W2l_ps[:], W2T, Wl, start=True, stop=True)
            W2l = sb.tile([HID, OUT], bf16, tag="W2l")
            nc.scalar.copy(W2l[:], W2l_ps[:])
            bb_ps = pm.tile([OUT, 1], f32, tag="ps")
            nc.tensor.matmul(bb_ps[:], Wl, b216, start=True, stop=True)
            bbl = sb.tile([OUT, 1], f32, tag="bbl")
            nc.scalar.activation(bbl[:], bb_ps[:], Ident, bias=bl)

            # ---- node conv 1 ----
            xW1_ps = pm.tile([N, HID], f32, tag="ps")
            nc.tensor.matmul(xW1_ps[:], xT[:], W1, start=True, stop=True)
            xW1 = sb.tile([N, HID], bf16, tag="xW1")
            nc.scalar.copy(xW1[:], xW1_ps[:])
            x1T_ps = pm.tile([HID, N], f32, tag="ps")
            nc.tensor.matmul(x1T_ps[:], xW1[:], d1[:], start=True, stop=True)
            x1T = sb.tile([HID, N], bf16, tag="x1T")
            nc.scalar.activation(x1T[:], x1T_ps[:], Relu, bias=b1)

            # ---- y = x1 @ W2 @ Wl (for the folded pool/head tail) ----
            y_ps = pm.tile([N, OUT], f32, tag="ps")
            nc.tensor.matmul(y_ps[:], x1T[:], W2l[:], start=True, stop=True)
            y16 = sb.tile([N, OUT], bf16, tag="y16")
            nc.scalar.copy(y16[:], y_ps[:])

            # ---- dv and the r / nsd tiles ----
            dvT_ps = pm.tile([N, 1], f32, tag="ps")
            nc.tensor.matmul(dvT_ps[:], x1T[:], peT, start=True, stop=True)
            dvr_ps = pm.tile([1, N], f32, tag="ps")
            nc.tensor.matmul(dvr_ps[:], peT, x1T[:], start=True, stop=True)

            dvT = sb.tile([N, 1], f32, tag="dvT")
            nc.scalar.copy(dvT[:], dvT_ps[:])
            c = sb.tile([N, 1], f32, tag="c")
            nc.vector.tensor_scalar(c[:], dvT_ps[:], 0.0, EPS, A.max, A.add)
            ric = sb.tile([N, 1], f32, tag="ric")
            nc.vector.reciprocal(ric[:], c[:])
            negdvT = sb.tile([N, 1], f32, tag="negdvT")
            nc.scalar.mul(negdvT[:], dvT_ps[:], -1.0)
            dvrow16 = sb.tile([1, N], bf16, tag="dvrow16")
            nc.scalar.copy(dvrow16[:], dvr_ps[:])
            ric16 = sb.tile([N, 1], bf16, tag="ric16")
            nc.vector.tensor_copy(ric16[:], ric[:])

            dvROW_ps = pm.tile([N, N], f32, tag="ps")
            nc.tensor.matmul(dvROW_ps[:], ones_row, dvrow16[:], start=True, stop=True)
            ricrow_ps = pm.tile([1, N], f32, tag="ps")
            nc.tensor.matmul(ricrow_ps[:], ric16[:], I116, start=True, stop=True)
            ricrow16 = sb.tile([1, N], bf16, tag="ricrow16")
            nc.vector.tensor_copy(ricrow16[:], ricrow_ps[:])
            ricROW_ps = pm.tile([N, N], f32, tag="ps")
            nc.tensor.matmul(
                ricROW_ps[:], ones_row, ricrow16[:], start=True, stop=True
            )

            # nsd[i,j] = -(dv_i + dv_j) on the scalar engine
            nsd16 = sb.tile([N, N], bf16, tag="nsd16")
            nc.scalar.activation(
                nsd16[:], dvROW_ps[:], Ident, bias=negdvT[:], scale=-1.0
            )
            # dvj16[i,j] = dv_j (for folding dv into the colsum row)
            dvj16 = sb.tile([N, N], bf16, tag="dvj16")
            nc.scalar.copy(dvj16[:], dvROW_ps[:])
            # r[i,j] = min(ric_i, ric_j)
            r16 = sb.tile([N, N], bf16, tag="r16")
            nc.vector.tensor_scalar(r16[:], ricROW_ps[:], ric[:], None, A.min)

            # ---- eR = relu(ea) on scalar ----
            eR = sb.tile([N, SLAB], bf16, tag="eR")
            nc.scalar.activation(eR[:], ea, Relu)

            def eRb(k):
                return eR[:, k * N:(k + 1) * N][:, None, :].to_broadcast(
                    [N, EDIM, N]
                )

            # ---- MAC products: P_k[i,(m,j)] = eR_k[i,j] * We[k,m] ----
            # P0-P3 vs host-expanded WeX (2x mode); P4 on gpsimd via the
            # svecB broadcast AP (1x, but concurrent).
            P = []
            for k in range(EDIM):
                Pk = sb.tile([N, SLAB], bf16, tag=f"P{k}", name=f"P{k}")
                P.append(Pk)
            WeB4 = svecB[:, 15 + 4 * 5:15 + 4 * 5 + 5][:, :, None].to_broadcast(
                [N, EDIM, N]
            )
            nc.gpsimd.tensor_tensor(m3(P[4][:]), eRb(4), WeB4, A.mult)
            nc.vector.tensor_tensor(m3(P[0][:]), eRb(0), m3(WeX(0)), A.mult)
            nc.vector.tensor_tensor(m3(P[1][:]), eRb(1), m3(WeX(1)), A.mult)
            nc.vector.tensor_tensor(m3(P[2][:]), eRb(2), m3(WeX(2)), A.mult)
            # adds regrouped so the (possibly late) P3 joins last
            A01 = sb.tile([N, SLAB], bf16, tag="A01")
            nc.vector.tensor_tensor(A01[:], P[0][:], P[1][:], A.add)
            A24 = sb.tile([N, SLAB], bf16, tag="A24")
            nc.vector.tensor_tensor(A24[:], P[2][:], P[4][:], A.add)
            nc.vector.tensor_tensor(m3(P[3][:]), eRb(3), m3(WeX(3)), A.mult)
            G1 = sb.tile([N, SLAB], bf16, tag="G1")
            nc.vector.tensor_tensor(G1[:], A01[:], A24[:], A.add)
            G = sb.tile([N, SLAB], bf16, tag="G")
            nc.vector.tensor_tensor(G[:], G1[:], P[3][:], A.add)

            # ---- chunked edge-conv pipeline: planes 0-1 then 2-4 ----
            Gn = sb.tile([N, SLAB], bf16, tag="Gn")
            Gd = sb.tile([N, SLAB], bf16, tag="Gd")
            q16 = sb.tile([N, SLAB], bf16, tag="q16")
            S = sb.tile([N, EDIM], f32, tag="S")
            t1b = sb.tile([N, EDIM], f32, tag="t1b")
            u16 = sb.tile([1, SLAB], bf16, tag="u16")
            e2 = sb.tile([N, SLAB], bf16, tag="e2")
            p2B = svecB[:, 5:10]

            srowA_ps = pw.tile([1, CH1[1] - CH1[0]], f32, tag="srowA_ps")
            srowB_ps = pw.tile([1, CH2[1] - CH2[0]], f32, tag="srowB_ps")
            srow_ps = [srowA_ps, srowB_ps]
            pU1 = pu.tile([N, CH1[1] - CH1[0]], f32, tag="pU1")
            pU2 = pu.tile([N, CH2[1] - CH2[0]], f32, tag="pU2")
            pU = [pU1, pU2]
            CHUNKS = [(0, 2, CH1), (2, 5, CH2)]

            def c3(ap, nm):
                return ap.rearrange("p (m j) -> p m j", m=nm)

            # stage 1: Gn + Gd per chunk; PE colsum + scalar u-copy follow
            for ci, (mlo, mhi, (clo, chi)) in enumerate(CHUNKS):
                nm = mhi - mlo
                sl = slice(clo, chi)
                nc.vector.tensor_tensor(
                    c3(Gn[:, sl], nm), c3(G[:, sl], nm),
                    r16[:][:, None, :].to_broadcast([N, nm, N]), A.mult,
                )
                nc.vector.tensor_tensor(
                    c3(Gd[:, sl], nm), c3(Gn[:, sl], nm),
                    dvj16[:][:, None, :].to_broadcast([N, nm, N]), A.mult,
                )
                nc.tensor.matmul(
                    srow_ps[ci][:], ones_col, Gd[:, sl], start=True, stop=True
                )
                nc.scalar.copy(u16[:, sl], srow_ps[ci][:])

            # stage 2: S/t1b for chunk A (e2#1 bias), then q + U/inject
            nc.vector.tensor_reduce(
                S[:, 0:2][:, :, None], c3(Gn[:, CH1[0]:CH1[1]], 2),
                mybir.AxisListType.X, A.add,
            )
            nc.vector.scalar_tensor_tensor(
                t1b[:, 0:2], S[:, 0:2], dvT[:, 0:1], beB[:, 0:2], A.mult, A.add
            )
            for ci, (mlo, mhi, (clo, chi)) in enumerate(CHUNKS):
                nm = mhi - mlo
                sl = slice(clo, chi)
                nc.vector.tensor_tensor(
                    c3(q16[:, sl], nm), c3(Gn[:, sl], nm),
                    nsd16[:][:, None, :].to_broadcast([N, nm, N]), A.mult,
                )
                nc.tensor.matmul(
                    pU[ci][:], ones_row, u16[:, sl], start=True, stop=False
                )
                nc.tensor.matmul(
                    pU[ci][:], I116, q16[:, sl], start=False, stop=True
                )
            nc.vector.tensor_reduce(
                S[:, 2:5][:, :, None], c3(Gn[:, CH2[0]:CH2[1]], 3),
                mybir.AxisListType.X, A.add,
            )
            nc.vector.scalar_tensor_tensor(
                t1b[:, 2:5], S[:, 2:5], dvT[:, 0:1], beB[:, 2:5], A.mult, A.add
            )

            # stage 3: e2 ACTs; rs accumulates on the PE per plane:
            # rs = sum_m p2_m rowsum(e2_m) = sum_m e2_m @ (p2_m ones)
            # (e2_m is symmetric, so rowsum == the colsum that rs needs)
            rs_ps = pm.tile([N, 1], f32, tag="ps")
            for ci, (mlo, mhi, (clo, chi)) in enumerate(CHUNKS):
                for m in range(mlo, mhi):
                    psl = slice(m * N, (m + 1) * N)
                    zsrc = pU[ci][:, (m - mlo) * N:(m - mlo + 1) * N]
                    nc.scalar.activation(
                        e2[:, psl], zsrc, Relu, bias=t1b[:, m:m + 1]
                    )
                    nc.tensor.matmul(
                        rs_ps[:], e2[:, psl], p2c16[:, m:m + 1],
                        start=(m == 0), stop=(m == EDIM - 1),
                    )

            # diag correction: dg_i = sum_m p2_m relu(2 dv_i S_im + be_m)
            # (computed early, off the critical path)
            dvT2 = sb.tile([N, 1], f32, tag="dvT2")
            nc.vector.tensor_scalar(dvT2[:], dvT[:], 2.0, None, A.mult)
            h1 = sb.tile([N, EDIM], f32, tag="h1")
            nc.vector.scalar_tensor_tensor(
                h1[:], S[:], dvT2[:, 0:1], beB, A.mult, A.add
            )
            h2 = sb.tile([N, EDIM], f32, tag="h2")
            nc.vector.tensor_scalar(h2[:], h1[:], 0.0, None, A.max)
            dgt = sb.tile([N, EDIM], f32, tag="dgt")
            nc.vector.tensor_tensor(dgt[:], h2[:], p2B, A.mult)
            dg = sb.tile([N, 1], f32, tag="dg")
            nc.vector.tensor_reduce(dg[:], dgt[:], mybir.AxisListType.X, A.add)

            # ---- tail: out = (1'(d2*mask) y)/N + bbl
            #      1'(d2*mask) = rs - diag-part = rs - dg ----
            rs16 = sb.tile([N, 1], bf16, tag="rs16")
            nc.vector.tensor_scalar(rs16[:], rs_ps[:], dg[:], None, A.subtract)
            out4_ps = pm.tile([OUT, 1], f32, tag="ps")
            nc.tensor.matmul(out4_ps[:], y16[:], rs16[:], start=True, stop=True)
            out_sb = sb.tile([OUT, 1], f32, tag="out_sb")
            nc.vector.scalar_tensor_tensor(
                out_sb[:], out4_ps[:], 1.0 / N, bbl[:], A.mult, A.add
            )
            nc.sync.dma_start(out=out_d[:], in_=out_sb[:])

    _split_excess_waits(nc, mybir)
    return nc


def _prep_inputs(inputs):
    import ml_dtypes

    bf = ml_dtypes.bfloat16
    ei = np.asarray(inputs["edge_index"][0], dtype=np.int64)
    ej = np.asarray(inputs["edge_index"][1], dtype=np.int64)
    ea = np.asarray(inputs["edge_attr"], dtype=np.float32)

    ea_dense = np.zeros((N, EDIM, N), dtype=np.float32)
    ea_dense[ei, :, ej] = ea
    ea_dense[ej, :, ei] = ea

    s16 = np.zeros((128, S16_W), dtype=bf)
    s16[0:N, C_EA:C_EA + SLAB] = ea_dense.reshape(N, SLAB).astype(bf)
    s16[0:ENC, C_ENCT:C_ENCT + N] = (
        np.asarray(inputs["encoding_raw"], dtype=np.float32).T.astype(bf)
    )
    s16[0:ENC, C_WENC:C_WENC + HID] = np.asarray(
        inputs["W_enc"], dtype=np.float32
    ).astype(bf)
    s16[0:HID, C_W1:C_W1 + HID] = np.asarray(inputs["W1"], np.float32).astype(bf)
    s16[0:HID, C_W2T:C_W2T + HID] = (
        np.asarray(inputs["W2"], np.float32).T.astype(bf)
    )
    s16[0:HID, C_WL:C_WL + OUT] = np.asarray(inputs["Wl"], np.float32).astype(bf)
    s16[0, C_SV:C_SV + 40] = np.concatenate(
        [
            np.asarray(inputs["p1"], np.float32).reshape(-1),
            np.asarray(inputs["p2"], np.float32).reshape(-1),
            np.asarray(inputs["be"], np.float32).reshape(-1),
            np.asarray(inputs["We"], np.float32).reshape(-1),
        ]
    ).astype(bf)
    s16[0, C_ONESR:C_ONESR + N] = np.ones(N, dtype=bf)
    s16[0:N, C_ONESC] = np.ones(N, dtype=bf)
    s16[0:HID, C_PET] = np.asarray(inputs["pe"], np.float32).reshape(-1).astype(bf)
    s16[0:HID, C_B2] = np.asarray(inputs["b2"], np.float32).reshape(-1).astype(bf)
    s16[0:N, C_I:C_I + N] = np.eye(N, dtype=np.float32).astype(bf)
    p2v = np.asarray(inputs["p2"], np.float32).reshape(-1)
    s16[0:N, C_MASK:C_MASK + EDIM] = np.broadcast_to(p2v[None, :], (N, EDIM)).astype(bf)
    We = np.asarray(inputs["We"], np.float32)  # [5,5] (k, m)
    for k in range(4):
        s16[0:N, C_WEX + k * SLAB:C_WEX + (k + 1) * SLAB] = np.broadcast_to(
            np.repeat(We[k], N)[None, :], (N, SLAB)
        ).astype(bf)

    s32 = np.zeros((128, S32_W), dtype=np.float32)
    svec = np.concatenate(
        [
            np.asarray(inputs["p1"], np.float32).reshape(-1),
            np.asarray(inputs["p2"], np.float32).reshape(-1),
            np.asarray(inputs["be"], np.float32).reshape(-1),
            np.asarray(inputs["We"], np.float32).reshape(-1),
        ]
    )
    s32[0:N, C32_SV:C32_SV + 40] = np.broadcast_to(svec[None, :], (N, 40))
    s32[0:HID, C32_BENC] = np.asarray(inputs["b_enc"], np.float32).reshape(-1)
    s32[0:HID, C32_B1] = np.asarray(inputs["b1"], np.float32).reshape(-1)
    s32[0:OUT, C32_BL] = np.asarray(inputs["bl"], np.float32).reshape(-1)

    return {"s16": s16, "s32": s32}


def kernel(**inputs) -> np.ndarray:
    import sys

    if "/opt/trn_rl_repo" not in sys.path:
        sys.path.insert(0, "/opt/trn_rl_repo")
    from concourse.bass_utils import run_bass_kernel_spmd

    if "nc" not in _CACHE:
        _CACHE["nc"] = _build()
    nc = _CACHE["nc"]

    in_map = _prep_inputs(inputs)
    res = run_bass_kernel_spmd(
        nc, [in_map] * 8, core_ids=list(range(8)), trace=False
    )
    return np.asarray(res.results[0]["out"], dtype=np.float32).reshape(1, OUT)

